# revision 51
# baseline (speedup 1.0000x reference)
"""GQA attention block (rmsnorm + qk-headnorm + rotary + softmax + out-proj)
for Trainium2, SPMD over 8 NeuronCores: 2-way data parallel (batch) x 4-way
tensor parallel (query-head groups). Partial out-proj results are summed on
host (the TP all-reduce).

Shapes (hardcoded): tokens [2,2048,2048] f32, rotary [2048,64], norm_w [2048],
Wq [2048,2048], Wkv [2048,1024], Wo [2048,2048], gamma_q [32,64], gamma_k [8,64].

Structure:
- tokens^T (token-tile-major), rstd(tokens), and rotary trig are prepared
  host-side; head-l2norm is scale invariant so the rmsnorm rstd is applied
  only to V. No on-device transposes or rmsnorm stats at all.
- Phase P: v/k/q projections, PE-dense. Each projection's l2-headnorm tail
  (sum-of-squares matmul -> sqrt -> reciprocal -> broadcast matmul) is split
  in two and emitted 1 and 2 tasks later so the PE never waits on the
  ACT/DVE chain; rotary for each head row fires as soon as its last slice
  normalizes, hidden under the remaining projections.
- Phase A: per (strip, pair): S matmuls (K=64 halves) -> one ACT exp per
  chunk over [128,2,512] psum (ACT is the near-saturated engine) -> ones-
  augmented AV matmuls lagging 2 chunks so in-order PE never head-of-line
  blocks on exp. Softmax normalization is DVE reciprocal + GPSIMD partition
  broadcast (no PE). The out-projection is decomposed into single-matmul
  fillers threaded one-per-chunk into the PE stream to absorb the exp-bound
  slack; the backlog drains PE-dense at the end. Output f16 partials.
"""

import sys

for _p in ("/opt/trn_rl_repo", "/opt/pypackages"):
    if _p not in sys.path:
        sys.path.append(_p)

from contextlib import ExitStack

import numpy as np

import concourse.bass as bass
import concourse.tile as tile
from concourse import bacc, bass_isa, mybir
from concourse.bass_utils import run_bass_kernel_spmd

B, N, DIM = 2, 2048, 2048
DH = 64  # head dim
QH, KVH = 32, 8
NCORES = 8
TPG = 4  # tensor-parallel groups
QH_PER = QH // TPG  # 8 q heads per core
NPAIR = QH_PER // 2  # 4 pairs of q heads packed 2-per-128-partitions
NT = N // 128  # 16 token tiles
NC = DIM // 128  # 16 contraction chunks
STRIP = 512  # q-strip width
NSTRIP = N // STRIP

F32 = mybir.dt.float32
F32R = mybir.dt.float32r
F16 = mybir.dt.float16

_BUILD = {}


def _build_nc():
    """Trace + compile the per-core Bass kernel (same program all cores)."""
    nc = bacc.Bacc(
        "TRN2", target_bir_lowering=False, debug=False, num_devices=NCORES
    )

    # xt layout: [p][tb][c][u]  (token-tile major so tiles stream in early)
    t_xt = nc.dram_tensor("xt", [128, NT * NC * 128], F16, kind="ExternalInput").ap()
    t_wq = nc.dram_tensor("wq", [128, NC * 512], F16, kind="ExternalInput").ap()
    t_wk = nc.dram_tensor("wk", [128, NC * 128], F16, kind="ExternalInput").ap()
    t_wv = nc.dram_tensor("wv", [128, NC * 128], F16, kind="ExternalInput").ap()
    t_wo = nc.dram_tensor("wo", [128, NPAIR * DIM], F16, kind="ExternalInput").ap()
    t_rstd = nc.dram_tensor("rstd", [128, NT], F32, kind="ExternalInput").ap()
    t_gq = nc.dram_tensor("gq", [128, NPAIR], F32, kind="ExternalInput").ap()
    t_gk = nc.dram_tensor("gk", [128, 1], F32, kind="ExternalInput").ap()
    t_sel2 = nc.dram_tensor("sel2", [128, 2], F16, kind="ExternalInput").ap()
    t_bc2 = nc.dram_tensor("bc2", [2, 128], F32R, kind="ExternalInput").ap()
    t_cos = nc.dram_tensor("cos_t", [128, N], F16, kind="ExternalInput").ap()
    t_sin = nc.dram_tensor("sins_t", [128, N], F16, kind="ExternalInput").ap()
    t_out = nc.dram_tensor("out", [N, DIM], F16, kind="ExternalOutput").ap()

    with tile.TileContext(nc) as tc, ExitStack() as ctx:
        persist = ctx.enter_context(tc.tile_pool(name="persist", bufs=1))
        big = ctx.enter_context(tc.tile_pool(name="big", bufs=1))

        zero_t = persist.tile([128, 1], F32)
        nc.vector.memset(zero_t, 0.0)
        tiny_t = persist.tile([128, 1], F32)
        nc.vector.memset(tiny_t, 1e-24)
        warm_t = persist.tile([128, 512], F16)
        nc.vector.memset(warm_t, 0.125)

        gq_sb = persist.tile([128, NPAIR], F32)
        gk_sb = persist.tile([128, 1], F32)
        rstd_sb = persist.tile([128, NT], F32)
        sel2_sb = persist.tile([128, 2], F16)
        bc2_sb = persist.tile([2, 128], F32R)

        # big persistent tensors
        xt_sb = big.tile([128, NT, NC, 128], F16, name="xt")
        wq_sb = big.tile([128, NC, 512], F16, name="wq")
        wk_sb = big.tile([128, NC, 128], F16, name="wk")
        wv_sb = big.tile([128, NC, 128], F16, name="wv")
        wo_sb = big.tile([128, NPAIR, DIM], F16, name="wo")
        cos_sb = big.tile([128, N], F16, name="cos")
        sin_sb = big.tile([128, N], F16, name="sin")
        qr_t = [big.tile([128, N], F16, name=f"qr{p}") for p in range(NPAIR)]
        kr_t = big.tile([128, N], F16, name="kr")
        # V tiles: [v_lo(64) | 1 | v_hi(64) | 1] per token tile
        v_all = big.tile([128, NT, 2 * DH + 2], F16, name="vall")
        nc.vector.memset(v_all[:, :, DH : DH + 1], 1.0)
        nc.vector.memset(v_all[:, :, 2 * DH + 1 : 2 * DH + 2], 1.0)
        o_t = [big.tile([128, N], F16, name=f"o{p}") for p in range(NPAIR)]

        # input DMAs, ordered so the earliest-needed tensors land first
        xt_r = t_xt.rearrange("p (t c u) -> p t c u", c=NC, u=128)
        nc.sync.dma_start(
            out=wk_sb[:], in_=t_wk.rearrange("p (c q) -> p c q", q=128)
        )
        nc.sync.dma_start(
            out=wv_sb[:], in_=t_wv.rearrange("p (c q) -> p c q", q=128)
        )
        for tb in range(4):
            nc.sync.dma_start(out=xt_sb[:, tb], in_=xt_r[:, tb])
        nc.sync.dma_start(out=gq_sb[:], in_=t_gq)
        nc.sync.dma_start(out=gk_sb[:], in_=t_gk)
        nc.sync.dma_start(out=rstd_sb[:], in_=t_rstd)
        nc.sync.dma_start(out=sel2_sb[:], in_=t_sel2)
        nc.sync.dma_start(out=bc2_sb[:], in_=t_bc2)
        nc.sync.dma_start(
            out=wq_sb[:], in_=t_wq.rearrange("p (c q) -> p c q", q=512)
        )
        for tb in range(4, 8):
            nc.sync.dma_start(out=xt_sb[:, tb], in_=xt_r[:, tb])
        nc.sync.dma_start(out=cos_sb[:], in_=t_cos)
        nc.sync.dma_start(out=sin_sb[:], in_=t_sin)
        for tb in range(8, NT):
            nc.sync.dma_start(out=xt_sb[:, tb], in_=xt_r[:, tb])
        nc.sync.dma_start(
            out=wo_sb[:], in_=t_wo.rearrange("p (h d) -> p h d", d=DIM)
        )

        # ---------------- phase P: projections + headnorm + rotary --------
        with ExitStack() as pp:
            qp_pool = pp.enter_context(tc.tile_pool(name="qpp", bufs=2, space="PSUM"))
            vp_pool = pp.enter_context(tc.tile_pool(name="vpp", bufs=2, space="PSUM"))
            hs_pool = pp.enter_context(tc.tile_pool(name="hsp", bufs=2, space="PSUM"))
            bp_pool = pp.enter_context(tc.tile_pool(name="bpp", bufs=2, space="PSUM"))
            hn_pool = pp.enter_context(tc.tile_pool(name="hnp", bufs=3))
            qn_pool = pp.enter_context(tc.tile_pool(name="qnp", bufs=1))
            sw_pool = pp.enter_context(tc.tile_pool(name="swp", bufs=2))

            # PE p-state warmup during the initial DMA wait
            junk = qp_pool.tile([128, 512], F32, tag="qps", name="junk")
            for i in range(6):
                nc.tensor.matmul(
                    junk[:], warm_t[:, 0:128], warm_t[:], start=(i == 0),
                    stop=(i == 5),
                )

            qn_t = [
                qn_pool.tile([128, N], F16, tag=f"qn{p}", name=f"qn{p}")
                for p in range(NPAIR)
            ]
            qn_k = qn_pool.tile([128, N], F16, tag="qnk", name="qnk")

            def headnorm_mm(proj_w, g_ap, qn, s):
                """Project slice s (16 MMs) + DVE evac/square; returns the
                two-stage normalization tail for deferred emission."""
                ssl = slice(s * 512, (s + 1) * 512)
                qps = qp_pool.tile([128, 512], F32, tag="qps", name="qps")
                for c in range(NC):
                    nc.tensor.matmul(
                        qps[:],
                        proj_w(c),
                        xt_sb[:, 4 * s : 4 * s + 4, c, :],
                        start=(c == 0), stop=(c == NC - 1),
                    )
                qsb = hn_pool.tile([128, 512], F16, tag="qsb")
                nc.vector.tensor_copy(qsb[:], qps[:])
                sq = hn_pool.tile([128, 512], F16, tag="sq")
                nc.vector.tensor_mul(sq[:], qsb[:], qsb[:])
                box = {}

                def tail1():
                    hsum = hs_pool.tile([2, 512], F32, tag="hs")
                    nc.tensor.matmul(
                        hsum[:], sel2_sb[:], sq[:], start=True, stop=True
                    )
                    hsq = hn_pool.tile([2, 512], F32R, tag="hsq")
                    nc.scalar.activation(
                        out=hsq[:], in_=hsum[:],
                        func=mybir.ActivationFunctionType.Sqrt, bias=tiny_t[0:2],
                    )
                    with nc.allow_low_precision(reason="f32r rstd"):
                        nc.vector.reciprocal(out=hsq[:], in_=hsq[:])
                    box["hsq"] = hsq

                def tail2():
                    bps = bp_pool.tile([128, 512], F32, tag="bp")
                    nc.tensor.matmul(
                        bps[:], bc2_sb[:], box["hsq"][:], start=True, stop=True
                    )
                    rsb = hn_pool.tile([128, 512], F16, tag="rsb")
                    nc.vector.tensor_scalar_mul(rsb[:], bps[:], g_ap)
                    nc.vector.tensor_mul(qn[:, ssl], qsb[:], rsb[:])

                return tail1, tail2

            def v_tile(tb):
                vps = vp_pool.tile([128, 128], F32, tag="vps")
                for c in range(NC):
                    nc.tensor.matmul(
                        vps[:], xt_sb[:, tb, c, :],
                        wv_sb[:, c, :], start=(c == 0), stop=(c == NC - 1),
                    )
                nc.vector.tensor_scalar_mul(
                    v_all[:, tb, 0:DH], vps[:, 0:DH], rstd_sb[:, tb : tb + 1]
                )
                nc.vector.tensor_scalar_mul(
                    v_all[:, tb, DH + 1 : 2 * DH + 1], vps[:, DH : 2 * DH],
                    rstd_sb[:, tb : tb + 1],
                )
                return None, None

            def rotary_finish(qn, dst):
                # rotate-half swap within each head (partition +-32)
                qsw = sw_pool.tile([128, N], F16, tag="qsw", name="qsw")
                for h0 in (0, 64):
                    nc.sync.dma_start(
                        out=qsw[h0 : h0 + 32, :], in_=qn[h0 + 32 : h0 + 64, :]
                    )
                    nc.sync.dma_start(
                        out=qsw[h0 + 32 : h0 + 64, :], in_=qn[h0 : h0 + 32, :]
                    )
                nc.vector.tensor_mul(qn[:], qn[:], cos_sb[:])
                nc.vector.tensor_mul(qsw[:], qsw[:], sin_sb[:])
                nc.vector.tensor_add(dst[:], qn[:], qsw[:])

            # task list: per strip [v x4, k, q0], then q1/q2/q3 rows; rotary
            # hooks fire right after the relevant row's last tail2
            tasks = []
            hooks = {}
            for s in range(NSTRIP):
                for tb in range(4 * s, 4 * s + 4):
                    if tb != 15:
                        tasks.append(lambda tb=tb: v_tile(tb))
                tasks.append(
                    lambda s=s: headnorm_mm(
                        lambda c: wk_sb[:, c, :], gk_sb[:], qn_k, s
                    )
                )
                tasks.append(
                    lambda s=s: headnorm_mm(
                        lambda c: wq_sb[:, c, 0:128],
                        gq_sb[:, 0:1], qn_t[0], s,
                    )
                )
            hooks[len(tasks) - 2] = lambda: rotary_finish(qn_k, kr_t)
            hooks[len(tasks) - 1] = lambda: rotary_finish(qn_t[0], qr_t[0])
            for p in range(1, NPAIR):
                for s in range(NSTRIP):
                    tasks.append(
                        lambda s=s, p=p: headnorm_mm(
                            lambda c, p=p: wq_sb[:, c, p * 128 : (p + 1) * 128],
                            gq_sb[:, p : p + 1], qn_t[p], s,
                        )
                    )
                hooks[len(tasks) - 1] = (
                    lambda p=p: rotary_finish(qn_t[p], qr_t[p])
                )
            tasks.append(lambda: v_tile(15))

            nt = len(tasks)
            t1s = [None] * nt
            t2s = [None] * nt
            for i, t in enumerate(tasks):
                t1s[i], t2s[i] = t()
                if i >= 1 and t1s[i - 1] is not None:
                    t1s[i - 1]()
                if i >= 2:
                    if t2s[i - 2] is not None:
                        t2s[i - 2]()
                    if i - 2 in hooks:
                        hooks[i - 2]()
            if nt >= 1 and t1s[nt - 1] is not None:
                t1s[nt - 1]()
            for j in (nt - 2, nt - 1):
                if j >= 0:
                    if t2s[j] is not None:
                        t2s[j]()
                    if j in hooks:
                        hooks[j]()

        # ---------------- phase A: attention + out-proj -------------------
        with ExitStack() as pa:
            sp_pool = pa.enter_context(tc.tile_pool(name="spp", bufs=3, space="PSUM"))
            o_pool = pa.enter_context(tc.tile_pool(name="opp", bufs=1, space="PSUM"))
            et_pool = pa.enter_context(tc.tile_pool(name="etp", bufs=4))
            nrm_pool = pa.enter_context(tc.tile_pool(name="nrm", bufs=2))
            osb_pool = pa.enter_context(tc.tile_pool(name="osb", bufs=2))

            # deferred emission queues: DVE/GPSIMD-only fillers run at chunk 0
            # (they must precede the next o-psum allocation); PE-bearing
            # fillers start at chunk 3, one per chunk
            dve_fill = []
            pe_fill = []

            def attention(p, st):
                ssl = slice(st * STRIP, (st + 1) * STRIP)
                state = {}

                def av(et, c):
                    nc.tensor.matmul(
                        state["olo"][:], v_all[:, c, 0 : DH + 1], et[:, 0, :],
                        start=(c == 0), stop=(c == NC - 1),
                    )
                    nc.tensor.matmul(
                        state["ohi"][:], v_all[:, c, DH + 1 : 2 * DH + 2],
                        et[:, 1, :],
                        start=(c == 0), stop=(c == NC - 1),
                    )

                pend = []
                for c in range(NC):
                    if c == 0:
                        while dve_fill:
                            dve_fill.pop(0)()
                    if c >= 3 and pe_fill:
                        pe_fill.pop(0)()
                    ck = slice(c * 128, (c + 1) * 128)
                    sps = sp_pool.tile([128, 2, STRIP], F32, tag="sps")
                    nc.tensor.matmul(
                        sps[:, 0, :], kr_t[0:DH, ck], qr_t[p][0:DH, ssl],
                        start=True, stop=True,
                    )
                    nc.tensor.matmul(
                        sps[:, 1, :], kr_t[DH:128, ck], qr_t[p][DH:128, ssl],
                        start=True, stop=True,
                    )
                    et = et_pool.tile([128, 2, STRIP], F16, tag="et")
                    nc.scalar.activation(
                        out=et[:], in_=sps[:],
                        func=mybir.ActivationFunctionType.Exp,
                        bias=zero_t[:], scale=float(DH) ** -0.5,
                    )
                    if c == 0:
                        state["olo"] = o_pool.tile(
                            [DH + 1, STRIP], F32, tag="olo", name="olo"
                        )
                        state["ohi"] = o_pool.tile(
                            [DH + 1, STRIP], F32, tag="ohi", name="ohi"
                        )
                    pend.append((et, c))
                    if len(pend) > 2:
                        av(*pend.pop(0))
                for e in pend:
                    av(*e)

                # softmax normalization: rows 0..63 divided by row 64, via
                # DVE reciprocal + GPSIMD partition broadcast (no PE)
                def norm():
                    for half, ops in ((0, state["olo"]), (1, state["ohi"])):
                        ocp = nrm_pool.tile(
                            [DH + 1, STRIP], F16, tag="ocp", name="ocp"
                        )
                        nc.vector.tensor_copy(ocp[:], ops[:])
                        den = nrm_pool.tile([1, STRIP], F16, tag="den", name="den")
                        with nc.allow_low_precision(reason="f16 den"):
                            nc.vector.reciprocal(
                                out=den[:], in_=ocp[DH : DH + 1, :]
                            )
                        rbc = nrm_pool.tile([DH, STRIP], F16, tag="rbc", name="rbc")
                        nc.gpsimd.partition_broadcast(rbc[:], den[:], channels=DH)
                        nc.vector.tensor_mul(
                            o_t[p][half * DH : (half + 1) * DH, ssl],
                            ocp[0:DH, :], rbc[:],
                        )

                dve_fill.append(norm)

            def outproj_h(tb, cs, box, half):
                def f():
                    if cs == 0 and half == 0:
                        box["osb"] = osb_pool.tile(
                            [128, DIM], F16, tag="osb", name="osb"
                        )
                    if half == 0:
                        box["xps"] = sp_pool.tile(
                            [128, 512], F32, tag="sps", name="xps"
                        )
                    nc.tensor.matmul(
                        box["xps"][:],
                        o_t[half][:, tb * 128 : (tb + 1) * 128],
                        wo_sb[:, half, cs * 512 : (cs + 1) * 512],
                        start=(half == 0), stop=(half == NPAIR - 1),
                    )
                    if half == NPAIR - 1:
                        nc.vector.tensor_copy(
                            box["osb"][:, cs * 512 : (cs + 1) * 512], box["xps"][:]
                        )
                        if cs == 3:
                            nc.sync.dma_start(
                                out=t_out[tb * 128 : (tb + 1) * 128, :],
                                in_=box["osb"][:],
                            )

                return f

            for st in range(NSTRIP):
                for p in range(NPAIR):
                    attention(p, st)
                for tb in range(4 * st, 4 * st + 4):
                    box = {}
                    for cs in range(4):
                        for p in range(NPAIR):
                            pe_fill.append(outproj_h(tb, cs, box, p))
            while dve_fill:
                dve_fill.pop(0)()
            while pe_fill:
                pe_fill.pop(0)()

    nc.compile()
    return nc


def _core_inputs(core, tokens, rotary, norm_w, Wq, Wkv, Wo, gamma_q, gamma_k):
    b, g = core // TPG, core % TPG
    # pair p = (8g+p, 8g+4+p); lo half -> kv head 2g, hi -> 2g+1
    heads = []
    for p in range(NPAIR):
        heads += [QH_PER * g + p, QH_PER * g + NPAIR + p]
    cols = np.concatenate([np.arange(h * DH, (h + 1) * DH) for h in heads])

    nw = norm_w[:, None].astype(np.float32)

    def chunked(w, q):  # [2048, q] -> [128, NC*q] with [p][c][q] layout
        return np.ascontiguousarray(
            w.reshape(NC, 128, q).transpose(1, 0, 2).reshape(128, NC * q)
        )

    wq = chunked((Wq * nw)[:, cols].astype(np.float16), 512)
    kcols = np.arange(2 * g * DH, (2 * g + 2) * DH)
    wk = chunked((Wkv[:, : KVH * DH] * nw)[:, kcols].astype(np.float16), 128)
    wv = chunked((Wkv[:, KVH * DH :] * nw)[:, kcols].astype(np.float16), 128)
    wo = np.ascontiguousarray(
        Wo[cols, :].astype(np.float16)
        .reshape(NPAIR, 128, DIM).transpose(1, 0, 2).reshape(128, NPAIR * DIM)
    )
    # xt layout [p][tb][c][u]: element (d=c*128+p, t=tb*128+u)
    xtT = np.ascontiguousarray(tokens[b].T).astype(np.float16)
    xt = np.ascontiguousarray(
        xtT.reshape(NC, 128, NT, 128).transpose(1, 2, 0, 3).reshape(128, -1)
    )

    var = np.mean(np.square(tokens[b].astype(np.float32)), axis=-1)
    rstd = 1.0 / np.sqrt(var + np.finfo(np.float32).eps)  # [N]
    rstd_h = np.ascontiguousarray(rstd.reshape(NT, 128).T.astype(np.float32))

    gq = np.empty((128, NPAIR), np.float32)
    for p in range(NPAIR):
        gq[:DH, p] = (gamma_q[heads[2 * p]] + 1.0) * DH**0.5
        gq[DH:, p] = (gamma_q[heads[2 * p + 1]] + 1.0) * DH**0.5
    gk = np.empty((128, 1), np.float32)
    gk[:DH, 0] = (gamma_k[2 * g] + 1.0) * DH**0.5
    gk[DH:, 0] = (gamma_k[2 * g + 1] + 1.0) * DH**0.5

    cosT = np.cos(rotary).T.astype(np.float32)  # [64, N]
    sinT = np.sin(rotary).T.astype(np.float32)
    sinS = np.concatenate([-sinT[:32], sinT[32:]], axis=0)
    cos_t = np.tile(cosT, (2, 1)).astype(np.float16)
    sins_t = np.tile(sinS, (2, 1)).astype(np.float16)

    sel2 = np.zeros((128, 2), np.float16)
    sel2[:DH, 0] = 1
    sel2[DH:, 1] = 1
    bc2 = np.zeros((2, 128), np.float32)
    bc2[0, :DH] = 1
    bc2[1, DH:] = 1

    return {
        "xt": xt,
        "wq": wq,
        "wk": wk,
        "wv": wv,
        "wo": wo,
        "rstd": rstd_h,
        "gq": gq,
        "gk": gk,
        "cos_t": cos_t,
        "sins_t": sins_t,
        "sel2": sel2,
        "bc2": bc2,
    }


def kernel(tokens, rotary, norm_w, Wq, Wkv, Wo, gamma_q, gamma_k, _bench=None):
    if "nc" not in _BUILD:
        _BUILD["nc"] = _build_nc()
    nc = _BUILD["nc"]

    in_maps = [
        _core_inputs(c, tokens, rotary, norm_w, Wq, Wkv, Wo, gamma_q, gamma_k)
        for c in range(NCORES)
    ]
    kw = dict(_bench or {})
    res = run_bass_kernel_spmd(nc, in_maps, list(range(NCORES)), **kw)
    if _bench is not None:
        _BUILD["last"] = res

    out = np.empty((B, N, DIM), np.float32)
    for b in range(B):
        acc = res.results[b * TPG]["out"].astype(np.float32)
        for g in range(1, TPG):
            acc = acc + res.results[b * TPG + g]["out"].astype(np.float32)
        out[b] = acc
    return out


# revision 54
# speedup vs baseline: 1.0003x; 1.0003x over previous
"""GQA attention block (rmsnorm + qk-headnorm + rotary + softmax + out-proj)
for Trainium2, SPMD over 8 NeuronCores: 2-way data parallel (batch) x 4-way
tensor parallel (query-head groups). Partial out-proj results are summed on
host (the TP all-reduce).

Shapes (hardcoded): tokens [2,2048,2048] f32, rotary [2048,64], norm_w [2048],
Wq [2048,2048], Wkv [2048,1024], Wo [2048,2048], gamma_q [32,64], gamma_k [8,64].

Structure:
- tokens^T, rstd(tokens), and rotary trig are prepared host-side; since
  head-l2norm is scale invariant, the rmsnorm rstd only needs to be applied
  to V (per source token), so no on-device rmsnorm or transposes at all.
- Phase P: v/k/q projections (PE-dense) with l2-headnorm via GPSIMD
  partition_all_reduce and deferred two-stage tails; rotary emitted inline as
  soon as each head row completes so it hides under remaining projections.
- Phase A: per (strip, pair) attention: S matmuls -> ACT exp (the saturated
  engine) -> AV matmuls lagging 2 chunks; softmax normalization is DVE+GPSIMD
  only; per-strip out-projection interleaves into the PE stream as small
  fillers; output f16 partials.
"""

import sys

for _p in ("/opt/trn_rl_repo", "/opt/pypackages"):
    if _p not in sys.path:
        sys.path.append(_p)

from contextlib import ExitStack

import numpy as np

import concourse.bass as bass
import concourse.tile as tile
from concourse import bacc, bass_isa, mybir
from concourse.bass_utils import run_bass_kernel_spmd

B, N, DIM = 2, 2048, 2048
DH = 64  # head dim
QH, KVH = 32, 8
NCORES = 8
TPG = 4  # tensor-parallel groups
QH_PER = QH // TPG  # 8 q heads per core
NPAIR = QH_PER // 2  # 4 pairs of q heads packed 2-per-128-partitions
NT = N // 128  # 16 token tiles
NC = DIM // 128  # 16 contraction chunks
STRIP = 512  # q-strip width
NSTRIP = N // STRIP

F32 = mybir.dt.float32
F32R = mybir.dt.float32r
F16 = mybir.dt.float16

_BUILD = {}


def _build_nc():
    """Trace + compile the per-core Bass kernel (same program all cores)."""
    nc = bacc.Bacc(
        "TRN2", target_bir_lowering=False, debug=False, num_devices=NCORES
    )

    # xt layout: [p][tb][c][u]  (token-tile major so tiles stream in early)
    t_xt = nc.dram_tensor("xt", [128, NT * NC * 128], F16, kind="ExternalInput").ap()
    t_wq = nc.dram_tensor("wq", [128, NC * 512], F16, kind="ExternalInput").ap()
    t_wk = nc.dram_tensor("wk", [128, NC * 128], F16, kind="ExternalInput").ap()
    t_wv = nc.dram_tensor("wv", [128, NC * 128], F16, kind="ExternalInput").ap()
    t_wo = nc.dram_tensor("wo", [128, NPAIR * DIM], F16, kind="ExternalInput").ap()
    t_rstd = nc.dram_tensor("rstd", [128, NT], F32, kind="ExternalInput").ap()
    t_gq = nc.dram_tensor("gq", [128, NPAIR], F32, kind="ExternalInput").ap()
    t_gk = nc.dram_tensor("gk", [128, 1], F32, kind="ExternalInput").ap()
    t_sel2 = nc.dram_tensor("sel2", [128, 2], F16, kind="ExternalInput").ap()
    t_bc2 = nc.dram_tensor("bc2", [2, 128], F32R, kind="ExternalInput").ap()
    t_cos = nc.dram_tensor("cos_t", [128, N], F16, kind="ExternalInput").ap()
    t_sin = nc.dram_tensor("sins_t", [128, N], F16, kind="ExternalInput").ap()
    t_out = nc.dram_tensor("out", [N, DIM], F16, kind="ExternalOutput").ap()

    with tile.TileContext(nc) as tc, ExitStack() as ctx:
        persist = ctx.enter_context(tc.tile_pool(name="persist", bufs=1))
        big = ctx.enter_context(tc.tile_pool(name="big", bufs=1))

        zero_t = persist.tile([128, 1], F32)
        nc.vector.memset(zero_t, 0.0)
        tiny_t = persist.tile([128, 1], F32)
        nc.vector.memset(tiny_t, 1e-24)
        warm_t = persist.tile([128, 512], F16)
        nc.vector.memset(warm_t, 0.125)

        gq_sb = persist.tile([128, NPAIR], F32)
        gk_sb = persist.tile([128, 1], F32)
        rstd_sb = persist.tile([128, NT], F32)
        sel2_sb = persist.tile([128, 2], F16)
        bc2_sb = persist.tile([2, 128], F32R)

        # big persistent tensors
        xt_sb = big.tile([128, NT, NC, 128], F16, name="xt")
        wq_sb = big.tile([128, NC, 512], F16, name="wq")
        wk_sb = big.tile([128, NC, 128], F16, name="wk")
        wv_sb = big.tile([128, NC, 128], F16, name="wv")
        wo_sb = big.tile([128, NPAIR, DIM], F16, name="wo")
        cos_sb = big.tile([128, N], F16, name="cos")
        sin_sb = big.tile([128, N], F16, name="sin")
        qr_t = [big.tile([128, N], F16, name=f"qr{p}") for p in range(NPAIR)]
        kr_t = big.tile([128, N], F16, name="kr")
        # V tiles: [v_lo(64) | 1 | v_hi(64) | 1] per token tile
        v_all = big.tile([128, NT, 2 * DH + 2], F16, name="vall")
        nc.vector.memset(v_all[:, :, DH : DH + 1], 1.0)
        nc.vector.memset(v_all[:, :, 2 * DH + 1 : 2 * DH + 2], 1.0)
        o_t = [big.tile([128, N], F16, name=f"o{p}") for p in range(NPAIR)]

        # input DMAs, ordered so the earliest-needed tensors land first
        xt_r = t_xt.rearrange("p (t c u) -> p t c u", c=NC, u=128)
        nc.sync.dma_start(
            out=wk_sb[:], in_=t_wk.rearrange("p (c q) -> p c q", q=128)
        )
        nc.sync.dma_start(
            out=wv_sb[:], in_=t_wv.rearrange("p (c q) -> p c q", q=128)
        )
        for tb in range(4):
            nc.sync.dma_start(out=xt_sb[:, tb], in_=xt_r[:, tb])
        nc.sync.dma_start(out=gq_sb[:], in_=t_gq)
        nc.sync.dma_start(out=gk_sb[:], in_=t_gk)
        nc.sync.dma_start(out=rstd_sb[:], in_=t_rstd)
        nc.sync.dma_start(out=sel2_sb[:], in_=t_sel2)
        nc.sync.dma_start(out=bc2_sb[:], in_=t_bc2)
        nc.sync.dma_start(
            out=wq_sb[:], in_=t_wq.rearrange("p (c q) -> p c q", q=512)
        )
        for tb in range(4, 8):
            nc.sync.dma_start(out=xt_sb[:, tb], in_=xt_r[:, tb])
        nc.sync.dma_start(out=cos_sb[:], in_=t_cos)
        nc.sync.dma_start(out=sin_sb[:], in_=t_sin)
        for tb in range(8, NT):
            nc.sync.dma_start(out=xt_sb[:, tb], in_=xt_r[:, tb])
        nc.sync.dma_start(
            out=wo_sb[:], in_=t_wo.rearrange("p (h d) -> p h d", d=DIM)
        )

        # ---------------- phase P: projections + headnorm + rotary --------
        with ExitStack() as pp:
            qp_pool = pp.enter_context(tc.tile_pool(name="qpp", bufs=2, space="PSUM"))
            vp_pool = pp.enter_context(tc.tile_pool(name="vpp", bufs=2, space="PSUM"))
            hs_pool = pp.enter_context(tc.tile_pool(name="hsp", bufs=2, space="PSUM"))
            bp_pool = pp.enter_context(tc.tile_pool(name="bpp", bufs=2, space="PSUM"))
            hn_pool = pp.enter_context(tc.tile_pool(name="hnp", bufs=3))
            qn_pool = pp.enter_context(tc.tile_pool(name="qnp", bufs=1))
            sw_pool = pp.enter_context(tc.tile_pool(name="swp", bufs=2))

            # PE p-state warmup during the initial DMA wait
            junk = qp_pool.tile([128, 512], F32, tag="qps", name="junk")
            for i in range(6):
                nc.tensor.matmul(
                    junk[:], warm_t[:, 0:128], warm_t[:], start=(i == 0),
                    stop=(i == 5),
                )

            qn_t = [
                qn_pool.tile([128, N], F16, tag=f"qn{p}", name=f"qn{p}")
                for p in range(NPAIR)
            ]
            qn_k = qn_pool.tile([128, N], F16, tag="qnk", name="qnk")

            def headnorm_mm(proj_w, g_ap, qn, s):
                """Project slice s (16 MMs) + DVE evac/square; returns the
                two-stage normalization tail for deferred emission."""
                ssl = slice(s * 512, (s + 1) * 512)
                qps = qp_pool.tile([128, 512], F32, tag="qps", name="qps")
                for c in range(NC):
                    nc.tensor.matmul(
                        qps[:],
                        proj_w(c),
                        xt_sb[:, 4 * s : 4 * s + 4, c, :],
                        start=(c == 0), stop=(c == NC - 1),
                    )
                qsb = hn_pool.tile([128, 512], F16, tag="qsb")
                nc.vector.tensor_copy(qsb[:], qps[:])
                sq = hn_pool.tile([128, 512], F16, tag="sq")
                nc.vector.tensor_mul(sq[:], qsb[:], qsb[:])
                box = {}

                def tail1():
                    hsum = hs_pool.tile([2, 512], F32, tag="hs")
                    nc.tensor.matmul(
                        hsum[:], sel2_sb[:], sq[:], start=True, stop=True
                    )
                    hsq = hn_pool.tile([2, 512], F32R, tag="hsq")
                    nc.scalar.activation(
                        out=hsq[:], in_=hsum[:],
                        func=mybir.ActivationFunctionType.Sqrt, bias=tiny_t[0:2],
                    )
                    with nc.allow_low_precision(reason="f32r rstd"):
                        nc.vector.reciprocal(out=hsq[:], in_=hsq[:])
                    box["hsq"] = hsq

                def tail2():
                    bps = bp_pool.tile([128, 512], F32, tag="bp")
                    nc.tensor.matmul(
                        bps[:], bc2_sb[:], box["hsq"][:], start=True, stop=True
                    )
                    rsb = hn_pool.tile([128, 512], F16, tag="rsb")
                    nc.vector.tensor_scalar_mul(rsb[:], bps[:], g_ap)
                    nc.vector.tensor_mul(qn[:, ssl], qsb[:], rsb[:])

                return tail1, tail2

            def v_tile(tb, pool=None, tag="vps"):
                vps = (pool or vp_pool).tile([128, 128], F32, tag=tag, name="vps")
                for c in range(NC):
                    nc.tensor.matmul(
                        vps[:], xt_sb[:, tb, c, :],
                        wv_sb[:, c, :], start=(c == 0), stop=(c == NC - 1),
                    )
                nc.vector.tensor_scalar_mul(
                    v_all[:, tb, 0:DH], vps[:, 0:DH], rstd_sb[:, tb : tb + 1]
                )
                nc.vector.tensor_scalar_mul(
                    v_all[:, tb, DH + 1 : 2 * DH + 1], vps[:, DH : 2 * DH],
                    rstd_sb[:, tb : tb + 1],
                )
                return None, None

            def rotary_finish(qn, dst):
                # rotate-half swap within each head (partition +-32)
                qsw = sw_pool.tile([128, N], F16, tag="qsw", name="qsw")
                for h0 in (0, 64):
                    nc.sync.dma_start(
                        out=qsw[h0 : h0 + 32, :], in_=qn[h0 + 32 : h0 + 64, :]
                    )
                    nc.sync.dma_start(
                        out=qsw[h0 + 32 : h0 + 64, :], in_=qn[h0 : h0 + 32, :]
                    )
                nc.vector.tensor_mul(qn[:], qn[:], cos_sb[:])
                nc.vector.tensor_mul(qsw[:], qsw[:], sin_sb[:])
                nc.vector.tensor_add(dst[:], qn[:], qsw[:])

            # task list: per strip [v x4, k, q0], then q1/q2/q3 rows; rotary
            # hooks fire right after the relevant row's last tail2
            tasks = []
            hooks = {}
            for s in range(NSTRIP):
                for tb in range(4 * s, 4 * s + 4):
                    if tb < 8:
                        tasks.append(lambda tb=tb: v_tile(tb))
                tasks.append(
                    lambda s=s: headnorm_mm(
                        lambda c: wk_sb[:, c, :], gk_sb[:], qn_k, s
                    )
                )
                tasks.append(
                    lambda s=s: headnorm_mm(
                        lambda c: wq_sb[:, c, 0:128],
                        gq_sb[:, 0:1], qn_t[0], s,
                    )
                )
            hooks[len(tasks) - 2] = lambda: rotary_finish(qn_k, kr_t)
            hooks[len(tasks) - 1] = lambda: rotary_finish(qn_t[0], qr_t[0])
            for p in range(1, NPAIR):
                for s in range(NSTRIP):
                    tasks.append(
                        lambda s=s, p=p: headnorm_mm(
                            lambda c, p=p: wq_sb[:, c, p * 128 : (p + 1) * 128],
                            gq_sb[:, p : p + 1], qn_t[p], s,
                        )
                    )
                hooks[len(tasks) - 1] = (
                    lambda p=p: rotary_finish(qn_t[p], qr_t[p])
                )


            nt = len(tasks)
            t1s = [None] * nt
            t2s = [None] * nt
            for i, t in enumerate(tasks):
                t1s[i], t2s[i] = t()
                if i >= 1 and t1s[i - 1] is not None:
                    t1s[i - 1]()
                if i >= 2:
                    if t2s[i - 2] is not None:
                        t2s[i - 2]()
                    if i - 2 in hooks:
                        hooks[i - 2]()
            if nt >= 1 and t1s[nt - 1] is not None:
                t1s[nt - 1]()
            for j in (nt - 2, nt - 1):
                if j >= 0:
                    if t2s[j] is not None:
                        t2s[j]()
                    if j in hooks:
                        hooks[j]()

        # ---------------- phase A: attention + out-proj -------------------
        with ExitStack() as pa:
            sp_pool = pa.enter_context(tc.tile_pool(name="spp", bufs=3, space="PSUM"))
            o_pool = pa.enter_context(tc.tile_pool(name="opp", bufs=1, space="PSUM"))
            et_pool = pa.enter_context(tc.tile_pool(name="etp", bufs=4))
            nrm_pool = pa.enter_context(tc.tile_pool(name="nrm", bufs=2))
            osb_pool = pa.enter_context(tc.tile_pool(name="osb", bufs=2))

            # deferred emission queues: DVE/GPSIMD-only fillers run at chunk 0
            # (they must precede the next o-psum allocation); PE-bearing
            # fillers start at chunk 3, one per chunk
            dve_fill = []
            pe_fill = []
            for tb in range(8, NT):
                pe_fill.append(
                    lambda tb=tb: v_tile(tb, pool=sp_pool, tag="sps")
                )

            def attention(p, st):
                ssl = slice(st * STRIP, (st + 1) * STRIP)
                state = {}

                def av(et, c):
                    nc.tensor.matmul(
                        state["olo"][:], v_all[:, c, 0 : DH + 1], et[:, 0, :],
                        start=(c == 0), stop=(c == NC - 1),
                    )
                    nc.tensor.matmul(
                        state["ohi"][:], v_all[:, c, DH + 1 : 2 * DH + 2],
                        et[:, 1, :],
                        start=(c == 0), stop=(c == NC - 1),
                    )

                pend = []
                for c in range(NC):
                    if c == 0:
                        while dve_fill:
                            dve_fill.pop(0)()
                    if c >= 3 and pe_fill:
                        pe_fill.pop(0)()
                    ck = slice(c * 128, (c + 1) * 128)
                    sps = sp_pool.tile([128, 2, STRIP], F32, tag="sps")
                    nc.tensor.matmul(
                        sps[:, 0, :], kr_t[0:DH, ck], qr_t[p][0:DH, ssl],
                        start=True, stop=True,
                    )
                    nc.tensor.matmul(
                        sps[:, 1, :], kr_t[DH:128, ck], qr_t[p][DH:128, ssl],
                        start=True, stop=True,
                    )
                    et = et_pool.tile([128, 2, STRIP], F16, tag="et")
                    nc.scalar.activation(
                        out=et[:], in_=sps[:],
                        func=mybir.ActivationFunctionType.Exp,
                        bias=zero_t[:], scale=float(DH) ** -0.5,
                    )
                    if c == 0:
                        state["olo"] = o_pool.tile(
                            [DH + 1, STRIP], F32, tag="olo", name="olo"
                        )
                        state["ohi"] = o_pool.tile(
                            [DH + 1, STRIP], F32, tag="ohi", name="ohi"
                        )
                    pend.append((et, c))
                    if len(pend) > 2:
                        av(*pend.pop(0))
                for e in pend:
                    av(*e)

                # softmax normalization: rows 0..63 divided by row 64, via
                # DVE reciprocal + GPSIMD partition broadcast (no PE)
                def norm():
                    for half, ops in ((0, state["olo"]), (1, state["ohi"])):
                        ocp = nrm_pool.tile(
                            [DH + 1, STRIP], F16, tag="ocp", name="ocp"
                        )
                        nc.vector.tensor_copy(ocp[:], ops[:])
                        den = nrm_pool.tile([1, STRIP], F16, tag="den", name="den")
                        with nc.allow_low_precision(reason="f16 den"):
                            nc.vector.reciprocal(
                                out=den[:], in_=ocp[DH : DH + 1, :]
                            )
                        rbc = nrm_pool.tile([DH, STRIP], F16, tag="rbc", name="rbc")
                        nc.gpsimd.partition_broadcast(rbc[:], den[:], channels=DH)
                        nc.vector.tensor_mul(
                            o_t[p][half * DH : (half + 1) * DH, ssl],
                            ocp[0:DH, :], rbc[:],
                        )

                dve_fill.append(norm)

            def outproj_h(tb, cs, box, half):
                def f():
                    if cs == 0 and half == 0:
                        box["osb"] = osb_pool.tile(
                            [128, DIM], F16, tag="osb", name="osb"
                        )
                    if half == 0:
                        box["xps"] = sp_pool.tile(
                            [128, 512], F32, tag="sps", name="xps"
                        )
                    nc.tensor.matmul(
                        box["xps"][:],
                        o_t[half][:, tb * 128 : (tb + 1) * 128],
                        wo_sb[:, half, cs * 512 : (cs + 1) * 512],
                        start=(half == 0), stop=(half == NPAIR - 1),
                    )
                    if half == NPAIR - 1:
                        nc.vector.tensor_copy(
                            box["osb"][:, cs * 512 : (cs + 1) * 512], box["xps"][:]
                        )
                        if cs == 3:
                            nc.sync.dma_start(
                                out=t_out[tb * 128 : (tb + 1) * 128, :],
                                in_=box["osb"][:],
                            )

                return f

            for st in range(NSTRIP):
                for p in range(NPAIR):
                    attention(p, st)
                for tb in range(4 * st, 4 * st + 4):
                    box = {}
                    for cs in range(4):
                        for p in range(NPAIR):
                            pe_fill.append(outproj_h(tb, cs, box, p))
            while dve_fill:
                dve_fill.pop(0)()
            while pe_fill:
                pe_fill.pop(0)()

    nc.compile()
    return nc


def _core_inputs(core, tokens, rotary, norm_w, Wq, Wkv, Wo, gamma_q, gamma_k):
    b, g = core // TPG, core % TPG
    # pair p = (8g+p, 8g+4+p); lo half -> kv head 2g, hi -> 2g+1
    heads = []
    for p in range(NPAIR):
        heads += [QH_PER * g + p, QH_PER * g + NPAIR + p]
    cols = np.concatenate([np.arange(h * DH, (h + 1) * DH) for h in heads])

    nw = norm_w[:, None].astype(np.float32)

    def chunked(w, q):  # [2048, q] -> [128, NC*q] with [p][c][q] layout
        return np.ascontiguousarray(
            w.reshape(NC, 128, q).transpose(1, 0, 2).reshape(128, NC * q)
        )

    wq = chunked((Wq * nw)[:, cols].astype(np.float16), 512)
    kcols = np.arange(2 * g * DH, (2 * g + 2) * DH)
    wk = chunked((Wkv[:, : KVH * DH] * nw)[:, kcols].astype(np.float16), 128)
    wv = chunked((Wkv[:, KVH * DH :] * nw)[:, kcols].astype(np.float16), 128)
    wo = np.ascontiguousarray(
        Wo[cols, :].astype(np.float16)
        .reshape(NPAIR, 128, DIM).transpose(1, 0, 2).reshape(128, NPAIR * DIM)
    )
    # xt layout [p][tb][c][u]: element (d=c*128+p, t=tb*128+u)
    xtT = np.ascontiguousarray(tokens[b].T).astype(np.float16)
    xt = np.ascontiguousarray(
        xtT.reshape(NC, 128, NT, 128).transpose(1, 2, 0, 3).reshape(128, -1)
    )

    var = np.mean(np.square(tokens[b].astype(np.float32)), axis=-1)
    rstd = 1.0 / np.sqrt(var + np.finfo(np.float32).eps)  # [N]
    rstd_h = np.ascontiguousarray(rstd.reshape(NT, 128).T.astype(np.float32))

    gq = np.empty((128, NPAIR), np.float32)
    for p in range(NPAIR):
        gq[:DH, p] = (gamma_q[heads[2 * p]] + 1.0) * DH**0.5
        gq[DH:, p] = (gamma_q[heads[2 * p + 1]] + 1.0) * DH**0.5
    gk = np.empty((128, 1), np.float32)
    gk[:DH, 0] = (gamma_k[2 * g] + 1.0) * DH**0.5
    gk[DH:, 0] = (gamma_k[2 * g + 1] + 1.0) * DH**0.5

    cosT = np.cos(rotary).T.astype(np.float32)  # [64, N]
    sinT = np.sin(rotary).T.astype(np.float32)
    sinS = np.concatenate([-sinT[:32], sinT[32:]], axis=0)
    cos_t = np.tile(cosT, (2, 1)).astype(np.float16)
    sins_t = np.tile(sinS, (2, 1)).astype(np.float16)

    sel2 = np.zeros((128, 2), np.float16)
    sel2[:DH, 0] = 1
    sel2[DH:, 1] = 1
    bc2 = np.zeros((2, 128), np.float32)
    bc2[0, :DH] = 1
    bc2[1, DH:] = 1

    return {
        "xt": xt,
        "wq": wq,
        "wk": wk,
        "wv": wv,
        "wo": wo,
        "rstd": rstd_h,
        "gq": gq,
        "gk": gk,
        "cos_t": cos_t,
        "sins_t": sins_t,
        "sel2": sel2,
        "bc2": bc2,
    }


def kernel(tokens, rotary, norm_w, Wq, Wkv, Wo, gamma_q, gamma_k, _bench=None):
    if "nc" not in _BUILD:
        _BUILD["nc"] = _build_nc()
    nc = _BUILD["nc"]

    in_maps = [
        _core_inputs(c, tokens, rotary, norm_w, Wq, Wkv, Wo, gamma_q, gamma_k)
        for c in range(NCORES)
    ]
    kw = dict(_bench or {})
    res = run_bass_kernel_spmd(nc, in_maps, list(range(NCORES)), **kw)
    if _bench is not None:
        _BUILD["last"] = res

    out = np.empty((B, N, DIM), np.float32)
    for b in range(B):
        acc = res.results[b * TPG]["out"].astype(np.float32)
        for g in range(1, TPG):
            acc = acc + res.results[b * TPG + g]["out"].astype(np.float32)
        out[b] = acc
    return out
